# revision 1
# baseline (speedup 1.0000x reference)
"""Trainium2 Bass kernel for the sparse-attention scorer (nn_Attention_89120571392536).

Math (per batch row b, history step s):
    pre  = hist_b @ U_b + bias_b          U_b = (W1b - W1d) + diag(cand_b) @ W1c   [64, 32]
    h    = relu(pre)                      bias_b = cand_b @ (W1a + W1d) + b1       [32]
    sc   = h @ (W2/8), masked (s >= len_b -> NEG_INF/8)   (b2 dropped: softmax shift-invariant)
    w    = softmax(sc over s)
    out  = sum_s w * hist[b, s, :]

Sparsity: lens ~ U[0, 200).  Host sorts each core's 512 rows by len desc;
all DMA + compute extents are truncated per sorted chunk (graph compiled
per-extents, cached).  len=0 rows (all-masked -> uniform/200) are fixed up
on host.

Structure (per core, sorted order; grp = 128 b, chunk = 32 b, quad = 4 b,
group = 8 b), software-pipelined so scoring of grp g+1 overlaps the
softmax+wsum of grp g on the PE queue:
  scoring: one fp8 DoubleRow MM per quad: lhsT [128, 2, 128] block-diag U
           (contraction 256 = 4b x 64d), rhs [128, 2, s] -> psum [128 = 4b x 32h, s].
  relu:    per quad, psum -> bf16 sbuf, bias fused; split vector/scalar/gpsimd.
  W2:      8 accumulating bf16 MMs per chunk (K = 128 = 4b x 32h) -> [32, s].
  softmax: mask-copy C_MASK, reduce_max, exp(+accum), recip, mult.
  wsum:    transpose w -> [s, b]; per 8-b group a bf16 MM lhsT [s, 8] w cols,
           rhs [s, 512] hist (8b x 64d) -> psum [8, 512] diag strips; 4 groups
           per bank; bank -> SBUF -> one whole-tile DMA per grp; host extracts
           the diagonal strips.
"""

import sys

sys.path.insert(0, "/opt/trn_rl_repo")

import numpy as np
import ml_dtypes

from contextlib import ExitStack

import concourse.bass as bass
import concourse.bacc as bacc
import concourse.tile as tile
from concourse import mybir
from concourse.bass_utils import run_bass_kernel_spmd

BF16 = ml_dtypes.bfloat16
FP8 = ml_dtypes.float8_e4m3
F32 = np.float32

N_CORES = 8
B = 4096
S = 200
D = 64
H = 32
B_LOC = B // N_CORES          # 512
NEG_INF = -(2.0 ** 32) + 1.0
C_MASK = NEG_INF / (D ** 0.5)

dt = mybir.dt
Alu = mybir.AluOpType
Act = mybir.ActivationFunctionType
DR = mybir.MatmulPerfMode.DoubleRow

RELU_ENGINES = "vs"       # per-quad jj -> engine (v=vector, s=scalar)

_GRAPH_CACHE = {}


def _pad4(x):
    return max(8, int(-(-int(x) // 4) * 4))


def _extents(hisLens):
    """Shared (across cores) extents from sorted-descending lens."""
    lens = np.asarray(hisLens).reshape(N_CORES, B_LOC)
    order = np.argsort(-lens, axis=1, kind="stable")
    sorted_lens = np.take_along_axis(lens, order, axis=1)
    maxlens = sorted_lens.max(axis=0)                      # [512] nonincreasing
    S32 = tuple(min(S, _pad4(maxlens[32 * c])) for c in range(16))
    S8 = tuple(min(S32[k // 4], _pad4(maxlens[8 * k])) for k in range(64))
    return order, S32, S8


def _build_graph(S32, S8):
    S128 = tuple(S32[4 * g] for g in range(4))
    HT_OFF = np.cumsum([0] + [16 * s for s in S32]).tolist()   # per-chunk fp8 offsets
    MV_OFF = np.cumsum([0] + list(S128)).tolist()

    nc = bacc.Bacc(None, target_bir_lowering=False)

    htflat = nc.declare_dram_parameter("htflat", [128, HT_OFF[16]], dt.float8e4, isOutput=False)
    u3 = nc.declare_dram_parameter("u3", [128, 128 * 2 * 128], dt.float8e4, isOutput=False)
    hrA = nc.declare_dram_parameter("hrA", [128, 16, 32 * D], dt.bfloat16, isOutput=False)
    hrB = nc.declare_dram_parameter("hrB", [S - 128, 16, 32 * D], dt.bfloat16, isOutput=False)
    biasP = nc.declare_dram_parameter("biasP", [128, 128], dt.float32, isOutput=False)
    w2P = nc.declare_dram_parameter("w2P", [128, 4 * 2 * H], dt.float8e4, isOutput=False)
    id128 = nc.declare_dram_parameter("id128", [128, 128], dt.bfloat16, isOutput=False)
    minv = nc.declare_dram_parameter("minv", [128, MV_OFF[4]], dt.uint8, isOutput=False)
    scr = nc.declare_dram_parameter("scr", [4, 128, 4 * 512], dt.bfloat16, isOutput=True)

    with ExitStack() as ctx:
        tc = ctx.enter_context(tile.TileContext(nc))

        consts = ctx.enter_context(tc.tile_pool(name="consts", bufs=1))
        hra_pool = ctx.enter_context(tc.tile_pool(name="hra", bufs=3))
        hrb_pool = ctx.enter_context(tc.tile_pool(name="hrb", bufs=3))
        relu_pool = ctx.enter_context(tc.tile_pool(name="relu", bufs=6))
        sc_pool = ctx.enter_context(tc.tile_pool(name="scores", bufs=2))
        sm_pool = ctx.enter_context(tc.tile_pool(name="smax", bufs=2))
        wexp_pool = ctx.enter_context(tc.tile_pool(name="wexp", bufs=2))
        wt_pool = ctx.enter_context(tc.tile_pool(name="wt", bufs=2))
        osb_pool = ctx.enter_context(tc.tile_pool(name="osb", bufs=2))
        ph_pool = ctx.enter_context(tc.tile_pool(name="ph", bufs=3, space="PSUM"))
        psc_pool = ctx.enter_context(tc.tile_pool(name="psc", bufs=2, space="PSUM"))
        scr_pool = ctx.enter_context(tc.tile_pool(name="scr", bufs=1, space="PSUM"))
        pw_pool = ctx.enter_context(tc.tile_pool(name="pw", bufs=2, space="PSUM"))

        # ---- load order tuned for start latency: first chunk's slices first ----
        ht0 = consts.tile([128, HT_OFF[4]], dt.float8e4)
        nc.sync.dma_start(ht0[:, 0:HT_OFF[1]], htflat[:, 0:HT_OFF[1]])       # chunk 0
        u3v = u3.ap().rearrange("p (j t m) -> p j t m", j=128, t=2)
        u3t0 = consts.tile([128, 32, 2, 128], dt.float8e4)
        nc.scalar.dma_start(u3t0[:, 0:8, :, :], u3v[:, 0:8, :, :])           # chunk 0 quads
        nc.sync.dma_start(ht0[:, HT_OFF[1]:HT_OFF[4]], htflat[:, HT_OFF[1]:HT_OFF[4]])
        nc.scalar.dma_start(u3t0[:, 8:32, :, :], u3v[:, 8:32, :, :])
        w2t = consts.tile([128, 4, 2, H], dt.float8e4)
        nc.sync.dma_start(w2t[:], w2P.ap().rearrange("p (g t m) -> p g t m", g=4, t=2))
        biast = consts.tile([128, 128], dt.float32)
        nc.sync.dma_start(biast[:], biasP[:, :])
        idt = consts.tile([128, 128], dt.bfloat16)
        nc.sync.dma_start(idt[:], id128[:, :])
        mtile = consts.tile([128, MV_OFF[4]], dt.uint8)
        nc.sync.dma_start(mtile[:], minv[:, :])
        ctile = consts.tile([128, S], dt.float32)
        nc.vector.memset(ctile[:], 16.0 * C_MASK)

        u3t123 = consts.tile([128, 96, 2, 128], dt.float8e4)
        nc.gpsimd.dma_start(u3t123[:], u3v[:, 32:128, :, :])                 # grps 1-3
        ht123 = consts.tile([128, HT_OFF[16] - HT_OFF[4]], dt.float8e4)
        nc.sync.dma_start(ht123[:], htflat[:, HT_OFF[4]:HT_OFF[16]])

        state = {}      # per-grp tiles carried between pipeline stages
        pending_w2 = [None]   # one-chunk-delayed W2 stage

        def flush_w2(nxt):
            prev = pending_w2[0]
            pending_w2[0] = nxt
            if prev is None:
                return
            relus, sc_sb, c4p, Scp = prev
            psc = psc_pool.tile([H, Scp], dt.float32)
            for pr in range(4):
                nc.tensor.matmul(psc[:], lhsT=w2t[:, pr, :, :], rhs=relus[pr][:],
                                 start=(pr == 0), stop=(pr == 3), perf_mode=DR)
            nc.scalar.copy(sc_sb[32 * c4p:32 * (c4p + 1), 0:Scp], psc[:])

        def emit_softmax(g):
            Sg = S128[g]
            sc_sb, hra_g, hrb_g = state.pop(g)
            nc.vector.copy_predicated(sc_sb[:], mtile[:, MV_OFF[g]:MV_OFF[g] + Sg],
                                      ctile[:, 0:Sg])
            negmax = sm_pool.tile([128, 1], dt.float32, tag="negmax")
            nc.vector.reduce_max(negmax[:], sc_sb[:], axis=mybir.AxisListType.X, negate=True)
            nm16 = sm_pool.tile([128, 1], dt.float32, tag="nm16")
            nc.vector.tensor_scalar(nm16[:], negmax[:], 1.0 / 16.0, None, op0=Alu.mult)
            wexp = wexp_pool.tile([128, Sg], dt.bfloat16)
            rowsum = sm_pool.tile([128, 1], dt.float32, tag="rowsum")
            nc.scalar.activation(wexp[:], sc_sb[:], Act.Exp, bias=nm16[:], scale=1.0 / 16.0,
                                 accum_out=rowsum[:])
            rinv = sm_pool.tile([128, 1], dt.float32, tag="rinv")
            nc.vector.reciprocal(rinv[:], rowsum[:])
            wnrm = wexp_pool.tile([128, Sg], dt.bfloat16, tag="wnrm")
            nc.vector.tensor_scalar(wnrm[:], wexp[:], rinv[:], None, op0=Alu.mult)

            L1 = min(Sg, 128)
            pt1 = scr_pool.tile([L1, 128], dt.bfloat16, tag="pt")
            nc.tensor.transpose(pt1[:], wnrm[:, 0:L1], idt[:])
            wt1 = wt_pool.tile([L1, 128], dt.bfloat16, tag="wt1")
            nc.vector.tensor_copy(wt1[:], pt1[:])
            wt2 = None
            if Sg > 128:
                pt2 = scr_pool.tile([Sg - 128, 128], dt.bfloat16, tag="pt")
                nc.tensor.transpose(pt2[:], wnrm[:, 128:Sg], idt[:])
                wt2 = wt_pool.tile([Sg - 128, 128], dt.bfloat16, tag="wt2")
                nc.scalar.copy(wt2[:], pt2[:])
            state[(g, "w")] = (wt1, wt2, hra_g, hrb_g)

        def emit_wsum(g):
            wt1, wt2, hra_g, hrb_g = state.pop((g, "w"))
            osb = osb_pool.tile([128, 4, 512], dt.bfloat16)
            for c4 in range(4):
                c = 4 * g + c4
                pw = pw_pool.tile([128, 512], dt.float32)
                for u in range(4):
                    k = 4 * c + u
                    K1 = min(S8[k], 128)
                    K2 = S8[k] - K1 if S8[k] > 128 else 0
                    bl = 32 * c4 + 8 * u       # grp-local first b
                    dst = pw[32 * u:32 * u + 8, :]
                    nc.tensor.matmul(dst, lhsT=wt1[0:K1, bl:bl + 8],
                                     rhs=hra_g[0:K1, c4, 8 * u:8 * u + 8, :],
                                     start=True, stop=(K2 == 0), tile_position=(0, 32 * u))
                    if K2 > 0:
                        nc.tensor.matmul(dst, lhsT=wt2[0:K2, bl:bl + 8],
                                         rhs=hrb_g[0:K2, c4, 8 * u:8 * u + 8, :],
                                         start=False, stop=True, tile_position=(0, 32 * u))
                if c4 % 2 == 0:
                    nc.vector.tensor_copy(osb[:, c4, :], pw[:])
                else:
                    nc.scalar.copy(osb[:, c4, :], pw[:])
            eng = nc.sync if g % 2 == 0 else nc.scalar
            eng.dma_start(scr[g, :, :], osb[:].rearrange("p c n -> p (c n)"))

        def emit_scoring(g):
            Sg = S128[g]
            K1g = min(Sg, 128)
            hra_g = hra_pool.tile([K1g, 4, 32, D], dt.bfloat16)
            (nc.sync if g % 2 == 0 else nc.scalar).dma_start(
                hra_g[:], hrA[0:K1g, 4 * g:4 * g + 4, :].rearrange("s c (b d) -> s c b d", d=D))
            hrb_g = None
            if Sg > 128:
                hrb_g = hrb_pool.tile([Sg - 128, 4, 32, D], dt.bfloat16)
                (nc.gpsimd if g == 0 else nc.scalar).dma_start(
                    hrb_g[:], hrB[0:Sg - 128, 4 * g:4 * g + 4, :].rearrange("s c (b d) -> s c b d", d=D))

            ht_g = ht0 if g == 0 else ht123
            HT_BASE = 0 if g == 0 else HT_OFF[4]
            u3_g = u3t0 if g == 0 else u3t123
            JBASE = 0 if g == 0 else 32
            sc_sb = sc_pool.tile([128, Sg], dt.float32)

            for c4 in range(4):
                c = 4 * g + c4
                Sc = S32[c]
                O_c = HT_OFF[c] - HT_BASE
                relus = []
                ph2 = None
                rp = None
                for jj in range(8):
                    j = 8 * c + jj          # global quad index
                    if jj % 2 == 0:
                        ph2 = ph_pool.tile([128, 2, Sc], dt.float32)
                        rp = relu_pool.tile([128, 2, Sc], dt.float8e4, tag=f"r{(jj // 2) % 4}")
                        relus.append(rp)
                    dst = ph2[:, jj % 2, :]
                    rhs = ht_g[:, O_c + jj * 2 * Sc: O_c + (jj + 1) * 2 * Sc].rearrange(
                        "p (t s) -> p t s", t=2)
                    nc.tensor.matmul(
                        dst, lhsT=u3_g[:, j - JBASE, :, :], rhs=rhs,
                        start=True, stop=True, perf_mode=DR,
                    )
                    bias_ap = biast[:, j:j + 1]
                    eng = RELU_ENGINES[jj % len(RELU_ENGINES)]
                    if eng == "s":
                        nc.scalar.activation(rp[:, jj % 2, :], ph2[:, jj % 2, :], Act.Relu,
                                             bias=bias_ap, scale=1.0)
                    else:
                        nc.vector.tensor_scalar(rp[:, jj % 2, :], ph2[:, jj % 2, :], bias_ap, 0.0,
                                                op0=Alu.add, op1=Alu.max)
                flush_w2((relus, sc_sb, c4, Sc))
                if c4 == 1 and g >= 1:
                    emit_softmax(g - 1)    # W2(g-1) fully flushed at c4==0

            state[g] = (sc_sb, hra_g, hrb_g)

        # 3-deep software pipeline:
        # slot g: [wsum g-2][scoring g, W2 one chunk behind, softmax g-1 at mid]
        emit_scoring(0)
        emit_scoring(1)
        for g in (2, 3):
            emit_wsum(g - 2)
            emit_scoring(g)
        flush_w2(None)
        emit_softmax(3)
        emit_wsum(2)
        emit_wsum(3)

    if not nc.is_finalized():
        nc.finalize()
    return nc


def _host_prep(candidate_embedding, hist_embeddings, hisLens, attW1, attB1, attW2, attB2,
               order, S32, S8):
    W1a = attW1[0:D]
    W1b = attW1[D:2 * D]
    W1c = attW1[2 * D:3 * D]
    W1d = attW1[3 * D:4 * D]
    Wbd = (W1b - W1d).astype(F32)
    Wc = (W1a + W1d).astype(F32)
    scale = 1.0 / (D ** 0.5)
    W2o = (attW2[:, 0] * scale).astype(F32)            # [32] (b2 dropped)

    S128 = [S32[4 * g] for g in range(4)]
    MV = int(sum(S128))

    # block-diag W2*16 for the 4 DoubleRow chunk matmuls (2 quads per MM)
    W2o16 = W2o * 16.0
    w2h = np.zeros((4, H, 4, 2, H), dtype=F32)
    for q in range(4):
        for pr in range(4):
            for t in range(2):
                w2h[q, :, pr, t, 4 * (2 * pr + t) + q] = W2o16
    w2P = np.ascontiguousarray(w2h.reshape(128, 4 * 2 * H)).astype(FP8)
    id128_np = np.eye(128, dtype=BF16)

    in_maps = []
    for ci in range(N_CORES):
        sl = slice(ci * B_LOC, (ci + 1) * B_LOC)
        ordc = order[ci]
        cand_c = candidate_embedding[sl][ordc].astype(F32)        # [512, 64] sorted
        hist_c = hist_embeddings[sl][ordc]                        # [512, 200, 64] sorted
        lens_c = np.asarray(hisLens[sl])[ordc]

        hs8 = hist_c.astype(FP8)
        hsb = hist_c.astype(BF16)

        # scoring data: [64pp+d, chunk-flat (jj, t, s)]
        parts = []
        for c in range(16):
            blk = hs8[32 * c:32 * (c + 1)].reshape(8, 2, 2, S, D)     # [jj, pp, t, s, d]
            arr = blk[:, :, :, :S32[c], :].transpose(1, 4, 0, 2, 3)   # [pp, d, jj, t, s]
            parts.append(np.ascontiguousarray(arr).reshape(128, -1))
        htflat = np.concatenate(parts, axis=1)

        # hist rows for wsum: [s, chunk, 32b*64d]
        hsT = hsb.transpose(1, 0, 2)                                  # [200, 512, 64]
        hrA = np.ascontiguousarray(hsT[0:128]).reshape(128, 16, 32 * D)
        hrB = np.ascontiguousarray(hsT[128:S]).reshape(S - 128, 16, 32 * D)

        # per-quad folded U, block-diag DoubleRow layout [128, 128, 2, 128]
        U = (Wbd[None, :, :] + cand_c[:, :, None] * W1c[None, :, :]).astype(FP8)  # [512, 64, 32]
        u3h = np.zeros((2, D, 128, 2, 128), dtype=FP8)                # [pp, d, j, t, m]
        for pp in range(2):
            for t in range(2):
                q = 2 * pp + t
                # [d, j, h] <- U[4j + q]
                u3h[pp, :, :, t, H * q:H * (q + 1)] = U[4 * np.arange(128) + q].transpose(1, 0, 2)
        u3 = np.ascontiguousarray(u3h.reshape(128, 128 * 2 * 128))

        bias = (cand_c @ Wc + attB1).astype(F32)                      # [512, 32]
        biasP = np.ascontiguousarray(
            bias.reshape(128, 2, 2, H).transpose(1, 2, 3, 0).reshape(128, 128)
        )

        mv = np.zeros((128, MV), dtype=np.uint8)
        off = 0
        for g in range(4):
            mv[:, off:off + S128[g]] = (
                np.arange(S128[g])[None, :] >= lens_c[128 * g:128 * (g + 1), None]
            )
            off += S128[g]

        in_maps.append({
            "htflat": htflat, "u3": u3, "hrA": hrA, "hrB": hrB,
            "biasP": biasP, "w2P": w2P, "id128": id128_np, "minv": mv,
        })
    return in_maps


def run(inputs, trace=False):
    """Returns (output [4096, 64] f32, exec_time_ns or None)."""
    hisLens = np.asarray(inputs["hisLens"])
    order, S32, S8 = _extents(hisLens)
    key = (S32, S8)
    if key not in _GRAPH_CACHE:
        _GRAPH_CACHE.clear()
        _GRAPH_CACHE[key] = _build_graph(S32, S8)
    nc = _GRAPH_CACHE[key]

    in_maps = _host_prep(**inputs, order=order, S32=S32, S8=S8)
    res = run_bass_kernel_spmd(nc, in_maps, core_ids=list(range(N_CORES)), trace=trace)

    jd = np.arange(8)
    outp = np.empty((B, D), dtype=F32)
    for ci in range(N_CORES):
        sl = slice(ci * B_LOC, (ci + 1) * B_LOC)
        scrr = res.results[ci]["scr"].astype(F32)          # [4, 128, 2048]
        block = np.empty((B_LOC, D), dtype=F32)
        for g in range(4):
            v = scrr[g].reshape(128, 4, 8, D)              # [p, c, jblock, d]
            rows = 32 * np.arange(4)[:, None] + jd[None, :]        # [u, j]
            t = v[rows]                                    # [u, j, c, jblock, d]
            diag = t[:, jd, :, jd, :]                      # [j, u, c, d]
            block[128 * g:128 * (g + 1)] = (
                diag.transpose(2, 1, 0, 3).reshape(128, D)  # (c, u, j) order
            )
        sblock = np.empty((B_LOC, D), dtype=F32)
        sblock[order[ci]] = block
        outp[sl] = sblock

    # len==0 rows: reference softmax is uniform over all 200 steps
    zmask = hisLens == 0
    if zmask.any():
        outp[zmask] = np.asarray(inputs["hist_embeddings"])[zmask].mean(axis=1)
    return outp, res.exec_time_ns


def kernel(**inputs):
    out, _ = run(inputs, trace=False)
    return out



# revision 35
# speedup vs baseline: 1.2047x; 1.2047x over previous
"""Trainium2 Bass kernel for the sparse-attention scorer (nn_Attention_89120571392536).

Math (per batch row b, history step s):
    pre  = hist_b @ U_b + bias_b          U_b = (W1b - W1d) + diag(cand_b) @ W1c   [64, 32]
    h    = relu(pre)                      bias_b = cand_b @ (W1a + W1d) + b1       [32]
    sc   = h @ (W2/8), masked (s >= len_b -> NEG_INF/8)   (b2 dropped: softmax shift-invariant)
    w    = softmax(sc over s)
    out  = sum_s w * hist[b, s, :]

Sparsity: lens ~ U[0, 200).  Host sorts each core's 512 rows by len desc;
all DMA + compute extents are truncated per sorted 8-row group (graph
compiled per-extents, cached).  len=0 rows fixed up on host.

Scoring (v3): per group of 8 sorted rows, the 8 per-row U matrices are
packed into ONE [128, 128] stationary as a 2(row)x4(col) grid of
[64d, 32h] tiles; 8 tile_position matmuls run concurrently on the 16
sub-arrays, each streaming that row's hist^T [64, S8] fp8.  All 8 write
disjoint regions of one PSUM bank [128 = 4b x 32h, 2 = r, S8].
relu+bias: 2 ops per group (vector / scalar), fp8 out [128, 2, S8].
W2: one DoubleRow MM per group (block-diag w2, K = 256 = 8b x 32h),
4 groups accumulate into psc [32b, Sc] per chunk.
softmax: mask-copy C_MASK, reduce_max, exp(+accum), recip, mult.
wsum: transpose w -> [s, b]; per 8-b group a bf16 MM lhsT [s, 8] w cols,
rhs [s, 512] hist (8b x 64d) -> psum [8, 512] diag strips; 4 groups
per bank; bank -> SBUF -> one whole-tile DMA per grp; host extracts
the diagonal strips.
"""

import sys

sys.path.insert(0, "/opt/trn_rl_repo")

import numpy as np
import ml_dtypes

from contextlib import ExitStack

import concourse.bass as bass
import concourse.bacc as bacc
import concourse.tile as tile
from concourse import mybir
from concourse.bass_utils import run_bass_kernel_spmd

BF16 = ml_dtypes.bfloat16
FP8 = ml_dtypes.float8_e4m3
F32 = np.float32

N_CORES = 8
B = 4096
S = 200
D = 64
H = 32
B_LOC = B // N_CORES          # 512
NEG_INF = -(2.0 ** 32) + 1.0
C_MASK = NEG_INF / (D ** 0.5)

dt = mybir.dt
Alu = mybir.AluOpType
Act = mybir.ActivationFunctionType
DR = mybir.MatmulPerfMode.DoubleRow

_GRAPH_CACHE = {}


def _pad4(x):
    return max(8, int(-(-int(x) // 4) * 4))


def _extents(hisLens):
    """Shared (across cores) extents from sorted-descending lens."""
    lens = np.asarray(hisLens).reshape(N_CORES, B_LOC)
    order = np.argsort(-lens, axis=1, kind="stable")
    sorted_lens = np.take_along_axis(lens, order, axis=1)
    maxlens = sorted_lens.max(axis=0)                      # [512] nonincreasing
    S32 = tuple(min(S, _pad4(maxlens[32 * c])) for c in range(16))
    S8 = tuple(min(S32[k // 4], _pad4(maxlens[8 * k])) for k in range(64))
    return order, S32, S8


def _build_graph(S32, S8):
    S128 = tuple(S32[4 * g] for g in range(4))
    HT_OFF = np.cumsum([0] + [4 * s for s in S8]).tolist()     # per-group fp8 offsets
    MV_OFF = np.cumsum([0] + list(S128)).tolist()

    nc = bacc.Bacc(None, target_bir_lowering=False)

    ht3 = nc.declare_dram_parameter("ht3", [128, HT_OFF[64]], dt.float8e4, isOutput=False)
    up3 = nc.declare_dram_parameter("up3", [128, 64 * 128], dt.float8e4, isOutput=False)
    bias3 = nc.declare_dram_parameter("bias3", [128, 128], dt.float32, isOutput=False)
    w23 = nc.declare_dram_parameter("w23", [128, 4 * 2 * 32], dt.float8e4, isOutput=False)
    hrA = nc.declare_dram_parameter("hrA", [4, 128, 16 * 32 * D // 4], dt.bfloat16, isOutput=False)
    hrB = nc.declare_dram_parameter("hrB", [4, S - 128, 16 * 32 * D // 4], dt.bfloat16, isOutput=False)
    id128 = nc.declare_dram_parameter("id128", [128, 128], dt.bfloat16, isOutput=False)
    minv = nc.declare_dram_parameter("minv", [128, MV_OFF[4]], dt.uint8, isOutput=False)
    scr = nc.declare_dram_parameter("scr", [4, 128, 128], dt.float32, isOutput=True)

    with ExitStack() as ctx:
        tc = ctx.enter_context(tile.TileContext(nc))

        consts = ctx.enter_context(tc.tile_pool(name="consts", bufs=1))
        hra_pool = ctx.enter_context(tc.tile_pool(name="hra", bufs=3))
        hrb_pool = ctx.enter_context(tc.tile_pool(name="hrb", bufs=3))
        relu_pool = ctx.enter_context(tc.tile_pool(name="relu", bufs=10))
        sc_pool = ctx.enter_context(tc.tile_pool(name="scores", bufs=2))
        sm_pool = ctx.enter_context(tc.tile_pool(name="smax", bufs=2))
        wexp_pool = ctx.enter_context(tc.tile_pool(name="wexp", bufs=2))
        wt_pool = ctx.enter_context(tc.tile_pool(name="wt", bufs=2))
        osb_pool = ctx.enter_context(tc.tile_pool(name="osb", bufs=2))
        ps_pool = ctx.enter_context(tc.tile_pool(name="pscore", bufs=4, space="PSUM"))
        psc_pool = ctx.enter_context(tc.tile_pool(name="psc", bufs=1, space="PSUM"))
        scr_pool = ctx.enter_context(tc.tile_pool(name="scr", bufs=1, space="PSUM"))
        pw_pool = ctx.enter_context(tc.tile_pool(name="pw", bufs=2, space="PSUM"))

        # ---- load order tuned for start latency + per-queue deadlines ----
        # Aggregate DMA is HBM-bound (~270 GB/s across sync/scalar/gpsimd), so
        # balance bytes per queue and order by deadline; outputs last (HWDGE is
        # FIFO per queue, a waiting DMA blocks everything behind it).
        ht3t = consts.tile([128, HT_OFF[64]], dt.float8e4)
        nc.sync.dma_start(ht3t[:, 0:HT_OFF[4]], ht3[:, 0:HT_OFF[4]])         # grp0 chunk 0
        up3v = up3.ap().rearrange("p (g t m) -> p g t m", g=64, t=2)
        up3t = consts.tile([128, 64, 2, 64], dt.float8e4)
        nc.scalar.dma_start(up3t[:, 0:4, :, :], up3v[:, 0:4, :, :])          # grp0 chunk 0
        bias3t = consts.tile([128, 128], dt.float32)
        nc.scalar.dma_start(bias3t[:], bias3[:, :])
        w23t = consts.tile([128, 4, 2, 32], dt.float8e4)
        nc.scalar.dma_start(w23t[:], w23.ap().rearrange("p (q t m) -> p q t m", q=4, t=2))
        nc.sync.dma_start(ht3t[:, HT_OFF[4]:HT_OFF[10]], ht3[:, HT_OFF[4]:HT_OFF[10]])
        nc.scalar.dma_start(ht3t[:, HT_OFF[10]:HT_OFF[16]], ht3[:, HT_OFF[10]:HT_OFF[16]])
        nc.scalar.dma_start(up3t[:, 4:16, :, :], up3v[:, 4:16, :, :])
        idt = consts.tile([128, 128], dt.bfloat16)
        nc.scalar.dma_start(idt[:], id128[:, :])
        mtile = consts.tile([128, MV_OFF[4]], dt.uint8)
        nc.scalar.dma_start(mtile[:], minv[:, :])
        ctile = consts.tile([128, S], dt.float32)
        nc.vector.memset(ctile[:], 16.0 * C_MASK)

        nc.scalar.dma_start(up3t[:, 16:64, :, :], up3v[:, 16:64, :, :])      # grps 1-3
        nc.sync.dma_start(ht3t[:, HT_OFF[16]:HT_OFF[32]], ht3[:, HT_OFF[16]:HT_OFF[32]])
        nc.gpsimd.dma_start(ht3t[:, HT_OFF[32]:HT_OFF[64]], ht3[:, HT_OFF[32]:HT_OFF[64]])

        state = {}      # per-grp tiles carried between pipeline stages
        pending_w2 = [None]   # one-chunk-delayed W2 stage

        def flush_w2(nxt):
            prev = pending_w2[0]
            pending_w2[0] = nxt
            if prev is None:
                return
            relus, exts, sc_sb, c4p, Scp = prev
            psc = psc_pool.tile([H, Scp], dt.float32)
            for q in range(4):
                nc.tensor.matmul(psc[:, 0:exts[q]], lhsT=w23t[:, q, :, :], rhs=relus[q][:],
                                 start=(q == 0), stop=(q == 3), perf_mode=DR)
            nc.scalar.copy(sc_sb[32 * c4p:32 * (c4p + 1), 0:Scp], psc[:])

        def emit_softmax(g):
            Sg = S128[g]
            sc_sb, hra_g, hrb_g = state.pop(g)
            nc.vector.copy_predicated(sc_sb[:], mtile[:, MV_OFF[g]:MV_OFF[g] + Sg],
                                      ctile[:, 0:Sg])
            negmax = sm_pool.tile([128, 1], dt.float32, tag="negmax")
            nc.vector.reduce_max(negmax[:], sc_sb[:], axis=mybir.AxisListType.X, negate=True)
            nm16 = sm_pool.tile([128, 1], dt.float32, tag="nm16")
            nc.vector.tensor_scalar(nm16[:], negmax[:], 1.0 / 16.0, None, op0=Alu.mult)
            wexp = wexp_pool.tile([128, Sg], dt.bfloat16)
            rowsum = sm_pool.tile([128, 1], dt.float32, tag="rowsum")
            nc.scalar.activation(wexp[:], sc_sb[:], Act.Exp, bias=nm16[:], scale=1.0 / 16.0,
                                 accum_out=rowsum[:])
            rinv = sm_pool.tile([128, 1], dt.float32, tag="rinv")
            nc.vector.reciprocal(rinv[:], rowsum[:])
            wnrm = wexp_pool.tile([128, Sg], dt.bfloat16, tag="wnrm")
            nc.vector.tensor_scalar(wnrm[:], wexp[:], rinv[:], None, op0=Alu.mult)

            L1 = min(Sg, 128)
            pt1 = scr_pool.tile([L1, 128], dt.bfloat16, tag="pt")
            nc.tensor.transpose(pt1[:], wnrm[:, 0:L1], idt[:])
            wt1 = wt_pool.tile([L1, 128], dt.bfloat16, tag="wt1")
            nc.vector.tensor_copy(wt1[:], pt1[:])
            wt2 = None
            if Sg > 128:
                pt2 = scr_pool.tile([Sg - 128, 128], dt.bfloat16, tag="pt")
                nc.tensor.transpose(pt2[:], wnrm[:, 128:Sg], idt[:])
                wt2 = wt_pool.tile([Sg - 128, 128], dt.bfloat16, tag="wt2")
                nc.scalar.copy(wt2[:], pt2[:])
            state[(g, "w")] = (wt1, wt2, hra_g, hrb_g)

        def emit_wsum(g):
            # hist-as-weights: per b-pair, LDW [K, 128 = 2b x 64d] bf16 then a
            # 2-col MM streaming that pair's softmax weights -> out [128, 2].
            wt1, wt2, hra_g, hrb_g = state.pop((g, "w"))
            pw = pw_pool.tile([128, 128], dt.float32, padded_shape=[128, 512])
            for pg in range(64):               # grp-local pair
                c4, jj = pg // 16, pg % 16
                k = 16 * g + pg // 4
                K1 = min(S8[k], 128)
                K2 = S8[k] - K1 if S8[k] > 128 else 0
                dst = pw[:, 2 * pg:2 * pg + 2]
                nc.tensor.matmul(dst, lhsT=hra_g[0:K1, c4, 2 * jj:2 * jj + 2, :],
                                 rhs=wt1[0:K1, 2 * pg:2 * pg + 2],
                                 start=True, stop=(K2 == 0))
                if K2 > 0:
                    nc.tensor.matmul(dst, lhsT=hrb_g[0:K2, c4, 2 * jj:2 * jj + 2, :],
                                     rhs=wt2[0:K2, 2 * pg:2 * pg + 2],
                                     start=False, stop=True)
            oc = osb_pool.tile([128, 128], dt.float32)
            nc.vector.tensor_copy(oc[:], pw[:])
            eng = nc.sync if g % 2 == 0 else nc.scalar
            eng.dma_start(scr[g, :, :], oc[:])

        def emit_scoring(g):
            Sg = S128[g]
            K1g = min(Sg, 128)
            hra_g = hra_pool.tile([K1g, 4, 32, D], dt.bfloat16)
            (nc.sync if g % 2 == 0 else nc.scalar).dma_start(
                hra_g[:], hrA[g, 0:K1g, :].rearrange("s (c b d) -> s c b d", c=4, d=D))
            hrb_g = None
            if Sg > 128:
                hrb_g = hrb_pool.tile([Sg - 128, 4, 32, D], dt.bfloat16)
                (nc.gpsimd if g == 0 else nc.scalar).dma_start(
                    hrb_g[:], hrB[g, 0:Sg - 128, :].rearrange("s (c b d) -> s c b d", c=4, d=D))

            sc_sb = sc_pool.tile([128, Sg], dt.float32)

            for c4 in range(4):
                c = 4 * g + c4
                Sc = S32[c]
                relus = []
                exts = []
                for q in range(4):
                    k = 4 * c + q          # global group index
                    Sk = S8[k]
                    bank = ps_pool.tile([128, 2, Sk], dt.float32,
                                        padded_shape=[128, 2, 256])
                    for t in (0, 1):
                        for cc in range(4):
                            u, j = cc // 2, cc % 2
                            o = HT_OFF[k] + (2 * t + j) * Sk
                            nc.tensor.matmul(
                                bank[32 * cc:32 * (cc + 1), t, :],
                                lhsT=up3t[64 * u:64 * (u + 1), k, t, 32 * j:32 * (j + 1)],
                                rhs=ht3t[64 * u:64 * (u + 1), o:o + Sk],
                                start=True, stop=True,
                                tile_position=(64 * u, 32 * cc),
                            )
                    rp = relu_pool.tile([128, 2, Sk], dt.float8e4)
                    relus.append(rp)
                    exts.append(Sk)
                    nc.vector.tensor_scalar(rp[:, 0, :], bank[:, 0, :],
                                            bias3t[:, 2 * k:2 * k + 1], 0.0,
                                            op0=Alu.add, op1=Alu.max)
                    nc.scalar.activation(rp[:, 1, :], bank[:, 1, :], Act.Relu,
                                         bias=bias3t[:, 2 * k + 1:2 * k + 2], scale=1.0)
                flush_w2((relus, exts, sc_sb, c4, Sc))
                if c4 == 1 and g >= 1:
                    emit_softmax(g - 1)    # W2(g-1) fully flushed at c4==0

            state[g] = (sc_sb, hra_g, hrb_g)

        # 3-deep software pipeline:
        # slot g: [wsum g-2][scoring g, W2 one chunk behind, softmax g-1 at mid]
        emit_scoring(0)
        emit_scoring(1)
        emit_scoring(2)
        emit_wsum(0)
        emit_scoring(3)
        emit_wsum(1)
        flush_w2(None)
        emit_softmax(3)
        emit_wsum(2)
        emit_wsum(3)

    if not nc.is_finalized():
        nc.finalize()
    return nc


def _host_prep(candidate_embedding, hist_embeddings, hisLens, attW1, attB1, attW2, attB2,
               order, S32, S8):
    W1a = attW1[0:D]
    W1b = attW1[D:2 * D]
    W1c = attW1[2 * D:3 * D]
    W1d = attW1[3 * D:4 * D]
    Wbd = (W1b - W1d).astype(F32)
    Wc = (W1a + W1d).astype(F32)
    scale = 1.0 / (D ** 0.5)
    W2o = (attW2[:, 0] * scale).astype(F32)            # [32] (b2 dropped)

    S128 = [S32[4 * g] for g in range(4)]
    MV = int(sum(S128))

    # block-diag w2*16 for the per-group DoubleRow W2 MMs
    W2o16 = W2o * 16.0
    w23h = np.zeros((4, 32, 4, 2, 32), dtype=F32)      # [c, h, q, r, j]
    for q in range(4):
        for r in range(2):
            for c in range(4):
                w23h[c, :, q, r, 8 * q + 4 * r + c] = W2o16
    w23 = np.ascontiguousarray(w23h.reshape(128, 4 * 2 * 32)).astype(FP8)
    id128_np = np.eye(128, dtype=BF16)

    in_maps = []
    for ci in range(N_CORES):
        sl = slice(ci * B_LOC, (ci + 1) * B_LOC)
        ordc = order[ci]
        cand_c = candidate_embedding[sl][ordc].astype(F32)        # [512, 64] sorted
        hist_c = hist_embeddings[sl][ordc]                        # [512, 200, 64] sorted
        lens_c = np.asarray(hisLens[sl])[ordc]

        hs8 = hist_c.astype(FP8)
        hsb = hist_c.astype(BF16)

        # scoring data: [64u+d, group-flat (2t+j, s)]
        v = hs8.reshape(64, 2, 2, 2, S, D).transpose(2, 5, 0, 1, 3, 4)  # [u, d, g, t, j, s]
        v = np.ascontiguousarray(v).reshape(128, 64, 4, S)
        ht3 = np.concatenate(
            [v[:, k, :, :S8[k]].reshape(128, -1) for k in range(64)], axis=1)

        # hist rows for wsum, grp-major: [grp, s, 128b*64d] (contiguous per grp)
        hsT = hsb.transpose(1, 0, 2)                                  # [200, 512, 64]
        hrA = np.ascontiguousarray(
            hsT[0:128].reshape(128, 4, 128 * D).transpose(1, 0, 2))   # [4, 128, 8192]
        hrB = np.ascontiguousarray(
            hsT[128:S].reshape(S - 128, 4, 128 * D).transpose(1, 0, 2))

        # per-group packed U: up3[64u+d, g, t, 32j+h] = U[8g+4t+2u+j][d, h]
        U = (Wbd[None, :, :] + cand_c[:, :, None] * W1c[None, :, :]).astype(FP8)  # [512, 64, 32]
        u = U.reshape(64, 2, 2, 2, D, H).transpose(2, 4, 0, 1, 3, 5)  # [u, d, g, t, j, h]
        up3 = np.ascontiguousarray(u).reshape(128, 64 * 128)

        # bias3[32c+h, 2g+r] = bias[8g+4r+c][h]
        bias = (cand_c @ Wc + attB1).astype(F32)                      # [512, 32]
        b3 = bias.reshape(64, 2, 4, H).transpose(2, 3, 0, 1)          # [c, h, g, r]
        bias3 = np.ascontiguousarray(b3).reshape(128, 128)

        mv = np.zeros((128, MV), dtype=np.uint8)
        off = 0
        for g in range(4):
            mv[:, off:off + S128[g]] = (
                np.arange(S128[g])[None, :] >= lens_c[128 * g:128 * (g + 1), None]
            )
            off += S128[g]

        in_maps.append({
            "ht3": ht3, "up3": up3, "bias3": bias3, "w23": w23,
            "hrA": hrA, "hrB": hrB, "id128": id128_np, "minv": mv,
        })
    return in_maps


def run(inputs, trace=False):
    """Returns (output [4096, 64] f32, exec_time_ns or None)."""
    hisLens = np.asarray(inputs["hisLens"])
    order, S32, S8 = _extents(hisLens)
    key = (S32, S8)
    if key not in _GRAPH_CACHE:
        _GRAPH_CACHE.clear()
        _GRAPH_CACHE[key] = _build_graph(S32, S8)
    nc = _GRAPH_CACHE[key]

    in_maps = _host_prep(**inputs, order=order, S32=S32, S8=S8)
    res = run_bass_kernel_spmd(nc, in_maps, core_ids=list(range(N_CORES)), trace=trace)

    outp = np.empty((B, D), dtype=F32)
    for ci in range(N_CORES):
        sl = slice(ci * B_LOC, (ci + 1) * B_LOC)
        scrr = res.results[ci]["scr"].astype(F32)          # [4, 128, 128]
        block = np.empty((B_LOC, D), dtype=F32)
        for g in range(4):
            v = scrr[g]                                    # [2b x 64d, 2 x pair]
            block[128 * g + 0:128 * (g + 1):2] = v[0:64, 0::2].T
            block[128 * g + 1:128 * (g + 1):2] = v[64:128, 1::2].T
        sblock = np.empty((B_LOC, D), dtype=F32)
        sblock[order[ci]] = block
        outp[sl] = sblock

    # len==0 rows: reference softmax is uniform over all 200 steps
    zmask = hisLens == 0
    if zmask.any():
        outp[zmask] = np.asarray(inputs["hist_embeddings"])[zmask].mean(axis=1)
    return outp, res.exec_time_ns


def kernel(**inputs):
    out, _ = run(inputs, trace=False)
    return out
